# revision 7
# baseline (speedup 1.0000x reference)
"""Trainium2 Bass kernel for Gaussian-KDE logsumexp (nn_GaussianKernel).

out[n] = logsumexp_m( -0.5*||(y_n - x_m)/bw||^2 - Z ),  Z = D/2*log(2pi) + D*log(bw) + log(M)

On-device factorization (per query row n, data col m):
    A[n,m] = (y_n/bw^2) . x_m  +  c_m,       c_m = -||x_m||^2/(2 bw^2)   (host, fp64)
    out[n] = logsumexp_m A[n,m] + r_n,       r_n = -||y_n||^2/(2 bw^2) - Z (host, fp64)

y and x are quantized to bf16 once on the host; c_m / r_n are computed from the
quantized values, so the device result is the exact logsumexp of slightly
perturbed points (error ~1e-3 relative; tolerance 2e-2).  c_m rides into PSUM
as a K=2 rank-2 bf16 matmul (ones^T @ [c_hi; c_lo], hi/lo bf16 split keeps c
accurate to ~0.03).

Sharding: data-parallel over the 2048 query rows -> 8 cores x 256 rows
(2 M-tiles of 128 partitions), each core holds the full x (K=D=128).

Per core: warmup matmuls run during the input-DMA wait to lift the PE HAM
clock gate; per M-tile the bias pass (start=True) + bf16 y.x pass (stop=True)
fill a [128,2048] PSUM tile, then DVE does one negated row-max, ACT does one
exp with fused row-sum accumulation, and ln() is a bitwise log2 approximation
on DVE (err ~0.03), so the only ACT table is exp, preloaded by a dummy
activation at kernel start.
"""

import sys
from math import log, pi

import numpy as np
import ml_dtypes

sys.path.insert(0, "/opt/trn_rl_repo")

import concourse.bacc as bacc
import concourse.bass as bass
import concourse.mybir as mybir
import concourse.tile as tile
from concourse.bass_utils import run_bass_kernel_spmd

BW = 0.1
N_QUERY = 2048
N_DATA = 2048
DIM = 128
N_CORES = 8
SHARD = N_QUERY // N_CORES  # 256 query rows per core

Z_CONST = 0.5 * DIM * log(2.0 * pi) + DIM * log(BW) + log(float(N_DATA))

NM = 512                   # matmul free-dim (one fp32 PSUM bank)
M_TILES = SHARD // 128     # 2
N_WARMUP = 6               # PE warmup matmuls (N=512 each) during DMA wait

LN2 = 0.6931471805599453
# ln(S) ~= (int_bits(S) * 2^-23 - 127 + 0.0430357) * ln2
LOG_S1 = LN2 / (1 << 23)
LOG_S2 = (0.0430357 - 127.0) * LN2

_CACHE = {}


def _build_nc():
    f32 = mybir.dt.float32
    bf16 = mybir.dt.bfloat16
    i32 = mybir.dt.int32
    fx = mybir.ActivationFunctionType
    nc = bacc.Bacc("TRN2", target_bir_lowering=False, debug=False)

    xt = nc.dram_tensor("xt", [DIM, N_DATA], bf16, kind="ExternalInput")
    yt = nc.dram_tensor("yt", [DIM, SHARD], bf16, kind="ExternalInput")
    crow_d = nc.dram_tensor("crow", [2, N_DATA], bf16, kind="ExternalInput")
    rvec_d = nc.dram_tensor("rvec", [128, M_TILES], f32, kind="ExternalInput")
    out = nc.dram_tensor("out", [128, M_TILES], f32, kind="ExternalOutput")

    with tile.TileContext(nc) as tc:
        with (
            tc.tile_pool(name="io", bufs=1) as io,
            tc.tile_pool(name="psum", bufs=2, space=bass.MemorySpace.PSUM) as psum,
            tc.tile_pool(name="work", bufs=1) as work,
            tc.tile_pool(name="small", bufs=2) as small,
        ):
            # ---- constants for the rank-2 bias pass + PE warmup fodder
            # (emitted first so the PE warmup can start ASAP)
            ones2 = io.tile([2, 128], bf16, tag="ones2")
            nc.gpsimd.memset(ones2[:], 1.0)
            junk2 = io.tile([2, 512], bf16, tag="junk2")
            nc.gpsimd.memset(junk2[:], 0.0)

            # ---- dummy exp: pulls the ACT exp-table load off the critical path
            dmy = small.tile([1, 1], f32, tag="dmy")
            dmy2 = small.tile([1, 1], f32, tag="dmy2")
            nc.gpsimd.memset(dmy[:], 0.0)
            nc.scalar.activation(dmy2[:], dmy[:], fx.Exp)

            # ---- input DMAs (crow first: bias passes need only it; yt on
            # sync too — scalar-issued loads showed ~3us extra latency)
            crow = io.tile([2, N_DATA], bf16, tag="crow")
            nc.sync.dma_start(crow[:], crow_d[:])
            yt_sb = io.tile([DIM, SHARD], bf16, tag="yt")
            nc.sync.dma_start(yt_sb[:], yt[:])
            xt_sb = io.tile([DIM, N_DATA], bf16, tag="xt")
            for h in range(N_DATA // NM):
                nc.sync.dma_start(xt_sb[:, h * NM:(h + 1) * NM],
                                  xt[:, h * NM:(h + 1) * NM])
            rvec = io.tile([128, M_TILES], f32, tag="rvec")
            nc.scalar.dma_start(rvec[:], rvec_d[:])

            A = [psum.tile([128, N_DATA], f32, tag="A", name=f"A{mt}")
                 for mt in range(M_TILES)]

            # ---- PE warmup: garbage matmuls (overwritten by the bias pass)
            # keep the HAM activity window busy while input DMAs complete
            for w in range(N_WARMUP):
                nc.tensor.matmul(A[0][:, :512], ones2[:], junk2[:],
                                 start=True, stop=True)

            # ---- PE: per M-tile: rank-2 bias pass then bf16 main pass
            # (mt-major so mt0's max/exp chain starts while mt1 is on the PE)
            for mt in range(M_TILES):
                for h in range(N_DATA // NM):
                    nc.tensor.matmul(A[mt][:, h * NM:(h + 1) * NM],
                                     ones2[:],
                                     crow[:, h * NM:(h + 1) * NM],
                                     start=True, stop=False)
                for h in range(N_DATA // NM):
                    nc.tensor.matmul(A[mt][:, h * NM:(h + 1) * NM],
                                     yt_sb[:, mt * 128:(mt + 1) * 128],
                                     xt_sb[:, h * NM:(h + 1) * NM],
                                     start=False, stop=True)

            # ---- per M-tile: DVE row-max -> ACT exp(+accum) -> bit-log tail
            esc = work.tile([128, N_DATA], bf16, tag="esc")
            spack = small.tile([128, M_TILES], f32, tag="spack")
            osb = small.tile([128, M_TILES], f32, tag="osb")
            for mt in range(M_TILES):
                nmax = small.tile([128, 1], f32, tag="nmax", name=f"nmax{mt}")
                nc.vector.tensor_reduce(nmax[:], A[mt][:],
                                        axis=mybir.AxisListType.X,
                                        op=mybir.AluOpType.max, negate=True)
                radj = small.tile([128, 1], f32, tag="radj", name=f"radj{mt}")
                nc.vector.tensor_sub(radj[:], rvec[:, mt:mt + 1], nmax[:])
                nc.scalar.activation(esc[:], A[mt][:], fx.Exp,
                                     bias=nmax[:], scale=1.0,
                                     accum_out=spack[:, mt:mt + 1])
                # out = ln(S) - nmax + rvec';  ln(S) via bitwise log2
                sbits = small.tile([128, 1], f32, tag="sbits", name=f"sb{mt}")
                nc.vector.tensor_copy(sbits[:], spack[:, mt:mt + 1].bitcast(i32))
                nc.vector.tensor_scalar(osb[:, mt:mt + 1], sbits[:],
                                        LOG_S1, radj[:],
                                        op0=mybir.AluOpType.mult,
                                        op1=mybir.AluOpType.add)
                nc.sync.dma_start(out[:, mt:mt + 1], osb[:, mt:mt + 1])

    nc.compile()
    return nc


def make_in_maps(y, x):
    """Host-side prep: bf16 quantization + fp64 norm corrections."""
    y = np.asarray(y, dtype=np.float32)
    x = np.asarray(x, dtype=np.float32)

    xq = x.astype(ml_dtypes.bfloat16)                       # (M, D) bf16
    xt = np.ascontiguousarray(xq.T)                         # (D, M) bf16
    # c_m from the quantized x actually used on device; hi/lo bf16 split
    xq64 = xq.astype(np.float64)
    c = (-0.5 / (BW * BW)) * np.sum(xq64 * xq64, axis=1)    # (M,) fp64
    c_hi = c.astype(ml_dtypes.bfloat16)
    c_lo = (c - c_hi.astype(np.float64)).astype(ml_dtypes.bfloat16)
    crow = np.ascontiguousarray(np.stack([c_hi, c_lo]))     # (2, M) bf16

    in_maps = []
    for i in range(N_CORES):
        ysh = y[i * SHARD:(i + 1) * SHARD]
        ytq = (ysh.astype(np.float64) / (BW * BW)).astype(ml_dtypes.bfloat16)
        # effective y-hat = ytq * bw^2;  r_n = -||y-hat||^2/(2 bw^2) - Z
        yt64 = ytq.astype(np.float64)
        r = -0.5 * (BW * BW) * np.sum(yt64 * yt64, axis=1) - Z_CONST + LOG_S2
        rvec = np.ascontiguousarray(
            r.reshape(M_TILES, 128).T).astype(np.float32)   # (128, M_TILES)
        in_maps.append({
            "xt": xt,
            "yt": np.ascontiguousarray(ytq.T),              # (D, SHARD) bf16
            "crow": crow,
            "rvec": rvec,
        })
    return in_maps


def kernel(y, x):
    assert np.asarray(y).shape == (N_QUERY, DIM)
    assert np.asarray(x).shape == (N_DATA, DIM)

    if "nc" not in _CACHE:
        _CACHE["nc"] = _build_nc()
    nc = _CACHE["nc"]

    in_maps = make_in_maps(y, x)
    res = run_bass_kernel_spmd(nc, in_maps, core_ids=list(range(N_CORES)))
    # out[p, mt] holds query row mt*128+p of the core's shard
    return np.concatenate(
        [r["out"].T.reshape(-1) for r in res.results]).astype(np.float32)


# revision 18
# speedup vs baseline: 1.1492x; 1.1492x over previous
"""Trainium2 Bass kernel for Gaussian-KDE logsumexp (nn_GaussianKernel).

out[n] = logsumexp_m( -0.5*||(y_n - x_m)/bw||^2 - Z ),  Z = D/2*log(2pi) + D*log(bw) + log(M)

On-device factorization (per query row n, data col m):
    A[n,m] = (y_n/bw^2) . x_m  +  c_m,       c_m = -||x_m||^2/(2 bw^2)   (host, fp64)
    out[n] = logsumexp_m A[n,m] + r_n,       r_n = -||y_n||^2/(2 bw^2) - Z (host, fp64)

y and x are quantized to bf16 once on the host; c_m / r_n are computed from the
quantized values, so the device result is the exact logsumexp of slightly
perturbed points (error ~1e-3 relative; tolerance 2e-2).  c_m rides into PSUM
as K=2 rank-2 bf16 matmuls (ones^T @ [c_hi; c_lo]); the 4 bias matmuls of an
M-tile sit in 4 distinct PE row-groups (tile_position) so they run
concurrently (~1 bank instead of 4 banks of streaming time).

Sharding: data-parallel over the 2048 query rows -> 8 cores x 256 rows
(2 M-tiles of 128 partitions), each core holds the full x (K=D=128).

Per core: warmup matmuls run during the input-DMA wait to lift the PE HAM
clock gate; per M-tile the bias pass (start=True) + bf16 y.x pass (stop=True)
fill a [128,2048] PSUM tile, then DVE does one negated row-max, ACT does one
exp with fused row-sum accumulation, and ln() is a bitwise log2 approximation
on DVE, so the only ACT table is exp, preloaded by a dummy activation at
kernel start.  The [128,2] result is PE-transposed to [2,128] so the output
store is 2 big descriptors instead of 128 4-byte read-modify-writes (which
cost ~4.4us of completion latency on the critical tail).
"""

import sys
from math import log, pi

import numpy as np
import ml_dtypes

sys.path.insert(0, "/opt/trn_rl_repo")

import concourse.bacc as bacc
import concourse.bass as bass
import concourse.mybir as mybir
import concourse.tile as tile
from concourse.bass_utils import run_bass_kernel_spmd

BW = 0.1
N_QUERY = 2048
N_DATA = 2048
DIM = 128
N_CORES = 8
SHARD = N_QUERY // N_CORES  # 256 query rows per core

Z_CONST = 0.5 * DIM * log(2.0 * pi) + DIM * log(BW) + log(float(N_DATA))

NM = 512                   # matmul free-dim (one fp32 PSUM bank)
N_BANKS = N_DATA // NM     # 4
M_TILES = SHARD // 128     # 2
N_WARMUP = 6               # PE warmup matmuls (N=512 each) during DMA wait

LN2 = 0.6931471805599453
# ln(S) ~= (int_bits(S) * 2^-23 - 127 + 0.0430357) * ln2
LOG_S1 = LN2 / (1 << 23)
LOG_S2 = (0.0430357 - 127.0) * LN2

_CACHE = {}


def _build_nc():
    f32 = mybir.dt.float32
    bf16 = mybir.dt.bfloat16
    i32 = mybir.dt.int32
    fx = mybir.ActivationFunctionType
    nc = bacc.Bacc("TRN2", target_bir_lowering=False, debug=False)

    xt = nc.dram_tensor("xt", [DIM, N_DATA], bf16, kind="ExternalInput")
    yt = nc.dram_tensor("yt", [DIM, SHARD], bf16, kind="ExternalInput")
    # crow4[32*i + 0, j] = c_hi[i*512 + j]; crow4[32*i + 1, j] = c_lo[...]
    crow_d = nc.dram_tensor("crow", [128, NM], bf16, kind="ExternalInput")
    rvec_d = nc.dram_tensor("rvec", [128, M_TILES], f32, kind="ExternalInput")
    # out[4*mt + i, r] holds query row mt*128 + 32*i + r of the core's shard
    out = nc.dram_tensor("out", [4 * M_TILES, 32], f32, kind="ExternalOutput")

    with tile.TileContext(nc) as tc:
        with (
            tc.tile_pool(name="io", bufs=1) as io,
            tc.tile_pool(name="psum", bufs=2, space=bass.MemorySpace.PSUM) as psum,
            tc.tile_pool(name="work", bufs=1) as work,
            tc.tile_pool(name="small", bufs=2) as small,
        ):
            # ---- constants first so the PE warmup can start ASAP
            junk2 = io.tile([2, NM], bf16, tag="junk2")
            nc.gpsimd.memset(junk2[:], 0.0)
            ones_sb = io.tile([128, 128], bf16, tag="ones")
            nc.gpsimd.memset(ones_sb[:], 1.0)

            # ---- dummy exp: pulls the ACT exp-table load off the critical path
            dmy = small.tile([1, 1], f32, tag="dmy")
            dmy2 = small.tile([1, 1], f32, tag="dmy2")
            nc.gpsimd.memset(dmy[:], 0.0)
            nc.scalar.activation(dmy2[:], dmy[:], fx.Exp)

            # ---- input DMAs (crow first: bias passes need only it)
            crow = io.tile([128, NM], bf16, tag="crow")
            nc.sync.dma_start(crow[:], crow_d[:])
            yt_sb = io.tile([DIM, SHARD], bf16, tag="yt")
            nc.sync.dma_start(yt_sb[:], yt[:])
            # xt chunks split across both HWDGE queues so issue serialization
            # on the sync sequencer doesn't delay the later chunks' receipts
            xt_sb = io.tile([DIM, N_DATA], bf16, tag="xt")
            for h in range(2):
                nc.sync.dma_start(xt_sb[:, h * NM:(h + 1) * NM],
                                  xt[:, h * NM:(h + 1) * NM])
            rvec = io.tile([128, M_TILES], f32, tag="rvec")
            nc.scalar.dma_start(rvec[:], rvec_d[:])
            for h in range(2, N_BANKS):
                nc.scalar.dma_start(xt_sb[:, h * NM:(h + 1) * NM],
                                    xt[:, h * NM:(h + 1) * NM])

            A = [psum.tile([128, N_DATA], f32, tag="A", name=f"A{mt}")
                 for mt in range(M_TILES)]

            # ---- PE warmup: garbage matmuls (overwritten by the bias pass)
            # keep the HAM activity window busy while input DMAs complete
            for w in range(N_WARMUP):
                nc.tensor.matmul(A[0][:, :NM], ones_sb[0:2, :], junk2[:],
                                 start=True, stop=True)

            # ---- PE per M-tile: 4 concurrent rank-2 bias matmuls (one per
            # bank, in 4 distinct row-groups) then the bf16 y.x pass
            for mt in range(M_TILES):
                for h in range(N_BANKS):
                    nc.tensor.matmul(A[mt][:, h * NM:(h + 1) * NM],
                                     ones_sb[32 * h:32 * h + 2, :],
                                     crow[32 * h:32 * h + 2, :],
                                     start=True, stop=False,
                                     tile_position=(32 * h, 0))
                for h in range(N_BANKS):
                    nc.tensor.matmul(A[mt][:, h * NM:(h + 1) * NM],
                                     yt_sb[:, mt * 128:(mt + 1) * 128],
                                     xt_sb[:, h * NM:(h + 1) * NM],
                                     start=False, stop=True)

            # ---- per M-tile: DVE row-max -> ACT exp(+accum) -> bit-log tail
            esc = work.tile([128, N_DATA], bf16, tag="esc")
            spack = small.tile([128, M_TILES], f32, tag="spack")
            osb = small.tile([128, 32], f32, tag="osb")
            for mt in range(M_TILES):
                nmax = small.tile([128, 1], f32, tag="nmax", name=f"nmax{mt}")
                nc.vector.tensor_reduce(nmax[:], A[mt][:],
                                        axis=mybir.AxisListType.X,
                                        op=mybir.AluOpType.max, negate=True)
                radj = small.tile([128, 1], f32, tag="radj", name=f"radj{mt}")
                nc.vector.tensor_sub(radj[:], rvec[:, mt:mt + 1], nmax[:])
                nc.scalar.activation(esc[:], A[mt][:], fx.Exp,
                                     bias=nmax[:], scale=1.0,
                                     accum_out=spack[:, mt:mt + 1])
                # out = ln(S) - nmax + rvec';  ln(S) via bitwise log2
                sbits = small.tile([128, 1], f32, tag="sbits", name=f"sb{mt}")
                nc.vector.tensor_copy(sbits[:], spack[:, mt:mt + 1].bitcast(i32))
                nc.vector.tensor_scalar(osb[:, mt:mt + 1], sbits[:],
                                        LOG_S1, radj[:],
                                        op0=mybir.AluOpType.mult,
                                        op1=mybir.AluOpType.add)

            # ---- pack the [128,2] result onto 8 partitions with a DVE 32x32
            # stream transpose (same engine as the writers, so inherently
            # ordered) so the output store is 8 big descriptors instead of
            # 128 4-byte RMWs (~4us completion receipt on the critical tail).
            # t32[32*i + mt, r] = osb[32*i + r, mt]
            t32 = small.tile([128, 32], f32, tag="t32")
            nc.vector.transpose(t32[:], osb[:])
            nc.sync.dma_start(out[0:4, :], t32[0:97:32, :])
            nc.scalar.dma_start(out[4:8, :], t32[1:98:32, :])

    nc.compile()
    return nc


def make_in_maps(y, x):
    """Host-side prep: bf16 quantization + fp64 norm corrections."""
    y = np.asarray(y, dtype=np.float32)
    x = np.asarray(x, dtype=np.float32)

    xq = x.astype(ml_dtypes.bfloat16)                       # (M, D) bf16
    xt = np.ascontiguousarray(xq.T)                         # (D, M) bf16
    # c_m from the quantized x actually used on device; hi/lo bf16 split,
    # laid out per 512-col bank on partitions {32h, 32h+1}
    xq64 = xq.astype(np.float64)
    c = (-0.5 / (BW * BW)) * np.sum(xq64 * xq64, axis=1)    # (M,) fp64
    c_hi = c.astype(ml_dtypes.bfloat16)
    c_lo = (c - c_hi.astype(np.float64)).astype(ml_dtypes.bfloat16)
    crow = np.zeros((128, NM), dtype=ml_dtypes.bfloat16)
    for h in range(N_BANKS):
        crow[32 * h + 0] = c_hi[h * NM:(h + 1) * NM]
        crow[32 * h + 1] = c_lo[h * NM:(h + 1) * NM]

    in_maps = []
    for i in range(N_CORES):
        ysh = y[i * SHARD:(i + 1) * SHARD]
        ytq = (ysh.astype(np.float64) / (BW * BW)).astype(ml_dtypes.bfloat16)
        # effective y-hat = ytq * bw^2;  r_n = -||y-hat||^2/(2 bw^2) - Z
        yt64 = ytq.astype(np.float64)
        r = -0.5 * (BW * BW) * np.sum(yt64 * yt64, axis=1) - Z_CONST + LOG_S2
        rvec = np.ascontiguousarray(
            r.reshape(M_TILES, 128).T).astype(np.float32)   # (128, M_TILES)
        in_maps.append({
            "xt": xt,
            "yt": np.ascontiguousarray(ytq.T),              # (D, SHARD) bf16
            "crow": crow,
            "rvec": rvec,
        })
    return in_maps


def kernel(y, x):
    assert np.asarray(y).shape == (N_QUERY, DIM)
    assert np.asarray(x).shape == (N_DATA, DIM)

    if "nc" not in _CACHE:
        _CACHE["nc"] = _build_nc()
    nc = _CACHE["nc"]

    in_maps = make_in_maps(y, x)
    res = run_bass_kernel_spmd(nc, in_maps, core_ids=list(range(N_CORES)))
    # out[4*mt + i, r] holds query row mt*128 + 32*i + r -> flat is in order
    return np.concatenate(
        [r["out"].reshape(-1) for r in res.results]).astype(np.float32)


# revision 21
# speedup vs baseline: 1.2776x; 1.1117x over previous
"""Trainium2 Bass kernel for Gaussian-KDE logsumexp (nn_GaussianKernel).

out[n] = logsumexp_m( -0.5*||(y_n - x_m)/bw||^2 - Z ),  Z = D/2*log(2pi) + D*log(bw) + log(M)

On-device factorization (per query row n, data col m):
    A[n,m] = (y_n/bw^2) . x_m  +  c_m,       c_m = -||x_m||^2/(2 bw^2)   (host, fp64)
    out[n] = logsumexp_m A[n,m] + r_n,       r_n = -||y_n||^2/(2 bw^2) - Z (host, fp64)

y and x are quantized to bf16 once on the host; c_m / r_n are computed from the
quantized values, so the result is the exact logsumexp of slightly perturbed
points (error ~1e-3 relative; tolerance 2e-2).  c_m rides into PSUM as K=2
rank-2 bf16 matmuls (ones^T @ [c_hi; c_lo]); four bias matmuls at a time sit
in 4 distinct PE row-groups (tile_position) so they stream concurrently.

Sharding is 2-D: 4 query-shards x 2 data-halves over the 8 cores.  Core
c = 2*q + d handles query block q (512 rows = 4 M-tiles) against data half d
(1024 cols).  Each core returns per-tile partial (-rowmax, sum exp) pairs;
the host merges the two data-halves with an exact fp64 logsumexp merge and
adds r_n - so the device needs no ln() and no r/Z handling at all.

Per core: warmup matmuls run during the input-DMA wait to keep the PE clock
gate up; per M-tile the bias pass (start=True) + bf16 y.x pass (stop=True)
fill a [128,1024] PSUM tile, then DVE does one negated row-max (written
straight into the output pack), ACT does one exp with fused row-sum
accumulation (accumulator drained straight into the output pack).  The
[128,8] pack is rearranged onto 32 partitions with a DVE 32x32 stream
transpose so the output store is 4 DMAs of big descriptors instead of 128
4-byte read-modify-writes (~4us of completion latency).  The only ACT table
is exp, preloaded by a dummy activation at kernel start.
"""

import sys
from math import log, pi

import numpy as np
import ml_dtypes

sys.path.insert(0, "/opt/trn_rl_repo")

import concourse.bacc as bacc
import concourse.bass as bass
import concourse.mybir as mybir
import concourse.tile as tile
from concourse.bass_utils import run_bass_kernel_spmd

BW = 0.1
N_QUERY = 2048
N_DATA = 2048
DIM = 128
N_CORES = 8

N_QSHARDS = 4
N_DHALVES = 2
QSHARD = N_QUERY // N_QSHARDS      # 512 query rows per core
DHALF = N_DATA // N_DHALVES        # 1024 data cols per core
M_TILES = QSHARD // 128            # 4
NM = 512                           # matmul free-dim (one fp32 PSUM bank)
N_BANKS = DHALF // NM              # 2 banks per M-tile
N_WARMUP = 5                       # PE warmup matmuls during DMA wait

Z_CONST = 0.5 * DIM * log(2.0 * pi) + DIM * log(BW) + log(float(N_DATA))

_CACHE = {}


def _build_nc():
    f32 = mybir.dt.float32
    bf16 = mybir.dt.bfloat16
    fx = mybir.ActivationFunctionType
    nc = bacc.Bacc("TRN2", target_bir_lowering=False, debug=False)

    xt = nc.dram_tensor("xt", [DIM, DHALF], bf16, kind="ExternalInput")
    yt = nc.dram_tensor("yt", [DIM, QSHARD], bf16, kind="ExternalInput")
    # crow rows: (b0_hi, b1_hi, b0_hi, b1_hi) and (b0_lo, b1_lo, b0_lo, b1_lo)
    # land on SBUF partitions {0,32,64,96} and {1,33,65,97} respectively, so
    # row-group 32*i serves bank i%2 for two M-tiles at once.
    crow_hi_d = nc.dram_tensor("crow_hi", [4, NM], bf16, kind="ExternalInput")
    crow_lo_d = nc.dram_tensor("crow_lo", [4, NM], bf16, kind="ExternalInput")
    # out[8*i + c, r]  =  pack[32*i + r, c]   (c = 2*t + kind, see below)
    out = nc.dram_tensor("out", [32, 32], f32, kind="ExternalOutput")

    with tile.TileContext(nc) as tc:
        with (
            tc.tile_pool(name="io", bufs=1) as io,
            tc.tile_pool(name="psum", bufs=4, space=bass.MemorySpace.PSUM) as psum,
            tc.tile_pool(name="work", bufs=1) as work,
            tc.tile_pool(name="small", bufs=2) as small,
        ):
            # ---- constants first so the PE warmup can start ASAP
            junk2 = io.tile([2, NM], bf16, tag="junk2")
            nc.gpsimd.memset(junk2[:], 0.0)
            ones_sb = io.tile([128, 128], bf16, tag="ones")
            nc.gpsimd.memset(ones_sb[:], 1.0)

            # ---- dummy exp: pulls the ACT exp-table load off the critical path
            dmy = small.tile([1, 1], f32, tag="dmy")
            dmy2 = small.tile([1, 1], f32, tag="dmy2")
            nc.gpsimd.memset(dmy[:], 0.0)
            nc.scalar.activation(dmy2[:], dmy[:], fx.Exp)

            # ---- input DMAs (crow first: bias passes need only it; yt on the
            # scalar queue runs in parallel with the sync queue)
            crow = io.tile([128, NM], bf16, tag="crow")
            nc.sync.dma_start(crow[0:97:32, :], crow_hi_d[:])
            nc.sync.dma_start(crow[1:98:32, :], crow_lo_d[:])
            xt_sb = io.tile([DIM, DHALF], bf16, tag="xt")
            for h in range(N_BANKS):
                nc.sync.dma_start(xt_sb[:, h * NM:(h + 1) * NM],
                                  xt[:, h * NM:(h + 1) * NM])
            yt_sb = io.tile([DIM, QSHARD], bf16, tag="yt")
            nc.scalar.dma_start(yt_sb[:], yt[:])

            A = [psum.tile([128, DHALF], f32, tag="A", name=f"A{t}")
                 for t in range(M_TILES)]

            # ---- PE warmup: garbage matmuls (overwritten by the bias pass)
            for w in range(N_WARMUP):
                nc.tensor.matmul(A[0][:, :NM], ones_sb[0:2, :], junk2[:],
                                 start=True, stop=True)

            # pack[:, 2t] = -rowmax(A[t]);  pack[:, 2t+1] = sum exp(A[t]-max)
            pack = small.tile([128, 32], f32, tag="pack")
            esc = work.tile([128, DHALF], bf16, tag="esc")

            # ---- PE: per M-tile pair: 4 concurrent rank-2 bias matmuls
            # (tile t bank b in row-group 32*(2*(t%2)+b)), then y.x passes
            for tp in range(M_TILES // 2):
                for t in (2 * tp, 2 * tp + 1):
                    for b in range(N_BANKS):
                        g = 32 * (2 * (t % 2) + b)
                        nc.tensor.matmul(A[t][:, b * NM:(b + 1) * NM],
                                         ones_sb[g:g + 2, :],
                                         crow[g:g + 2, :],
                                         start=True, stop=False,
                                         tile_position=(g, 0))
                for t in (2 * tp, 2 * tp + 1):
                    for b in range(N_BANKS):
                        nc.tensor.matmul(A[t][:, b * NM:(b + 1) * NM],
                                         yt_sb[:, t * 128:(t + 1) * 128],
                                         xt_sb[:, b * NM:(b + 1) * NM],
                                         start=False, stop=True)
                    # DVE row-max and ACT exp+accum write the pack directly
                    nc.vector.tensor_reduce(pack[:, 2 * t:2 * t + 1], A[t][:],
                                            axis=mybir.AxisListType.X,
                                            op=mybir.AluOpType.max, negate=True)
                    nc.scalar.activation(esc[:], A[t][:], fx.Exp,
                                         bias=pack[:, 2 * t:2 * t + 1],
                                         scale=1.0,
                                         accum_out=pack[:, 2 * t + 1:2 * t + 2])

            # ---- rearrange the [128,8] pack onto 32 partitions and store
            # t32[32*i + c, r] = pack[32*i + r, c]
            t32 = small.tile([128, 32], f32, tag="t32")
            nc.vector.transpose(t32[:], pack[:])
            nc.sync.dma_start(out[0:8, :], t32[0:8, :])
            nc.sync.dma_start(out[8:16, :], t32[32:40, :])
            nc.scalar.dma_start(out[16:24, :], t32[64:72, :])
            nc.scalar.dma_start(out[24:32, :], t32[96:104, :])

    nc.compile()
    return nc


def make_in_maps(y, x):
    """Host-side prep: bf16 quantization + per-core shard layouts."""
    y = np.asarray(y, dtype=np.float32)
    x = np.asarray(x, dtype=np.float32)

    xq = x.astype(ml_dtypes.bfloat16)                       # (M, D) bf16
    xq64 = xq.astype(np.float64)
    c = (-0.5 / (BW * BW)) * np.sum(xq64 * xq64, axis=1)    # (M,) fp64
    c_hi = c.astype(ml_dtypes.bfloat16)
    c_lo = (c - c_hi.astype(np.float64)).astype(ml_dtypes.bfloat16)

    xts, crow_his, crow_los = [], [], []
    for d in range(N_DHALVES):
        sl = slice(d * DHALF, (d + 1) * DHALF)
        xts.append(np.ascontiguousarray(xq[sl].T))          # (D, DHALF)
        hi = c_hi[sl].reshape(N_BANKS, NM)                  # rows b0,b1
        lo = c_lo[sl].reshape(N_BANKS, NM)
        crow_his.append(np.ascontiguousarray(
            np.concatenate([hi, hi])))                      # (4, NM)
        crow_los.append(np.ascontiguousarray(np.concatenate([lo, lo])))

    in_maps = []
    for core in range(N_CORES):
        q, d = core // N_DHALVES, core % N_DHALVES
        ysh = y[q * QSHARD:(q + 1) * QSHARD]
        ytq = (ysh.astype(np.float64) / (BW * BW)).astype(ml_dtypes.bfloat16)
        in_maps.append({
            "xt": xts[d],
            "yt": np.ascontiguousarray(ytq.T),              # (D, QSHARD) bf16
            "crow_hi": crow_his[d],
            "crow_lo": crow_los[d],
        })
    return in_maps


def _unpack(res_out):
    """(32,32) device pack -> (nmax, s) arrays of shape (M_TILES, 128)."""
    # res_out[8*i + 2*t + k, r] = pack[32*i + r, 2*t + k], query t*128+32*i+r
    o = res_out.reshape(4, 8, 32).astype(np.float64)        # [i, c, r]
    nmax = np.empty((M_TILES, 128))
    s = np.empty((M_TILES, 128))
    for t in range(M_TILES):
        nmax[t] = o[:, 2 * t, :].reshape(128)               # i-major, then r
        s[t] = o[:, 2 * t + 1, :].reshape(128)
    return nmax, s


def assemble_output(results, y):
    """Host-side fp64 merge of the two data-halves + r_n - Z correction."""
    y = np.asarray(y, dtype=np.float32)
    out = np.empty(N_QUERY, dtype=np.float64)
    for q in range(N_QSHARDS):
        parts = []
        for d in range(N_DHALVES):
            nmax, s = _unpack(results[q * N_DHALVES + d]["out"])
            parts.append((-nmax) + np.log(s))               # partial logsumexp
        L = np.logaddexp(parts[0], parts[1]).reshape(-1)    # (512,)
        ysh = y[q * QSHARD:(q + 1) * QSHARD]
        ytq = (ysh.astype(np.float64) / (BW * BW)).astype(ml_dtypes.bfloat16)
        yt64 = ytq.astype(np.float64)
        r = -0.5 * (BW * BW) * np.sum(yt64 * yt64, axis=1) - Z_CONST
        out[q * QSHARD:(q + 1) * QSHARD] = L + r
    return out.astype(np.float32)


def kernel(y, x):
    y = np.asarray(y, dtype=np.float32)
    x = np.asarray(x, dtype=np.float32)
    assert y.shape == (N_QUERY, DIM) and x.shape == (N_DATA, DIM)

    if "nc" not in _CACHE:
        _CACHE["nc"] = _build_nc()
    nc = _CACHE["nc"]

    in_maps = make_in_maps(y, x)
    res = run_bass_kernel_spmd(nc, in_maps, core_ids=list(range(N_CORES)))
    return assemble_output(res.results, y)
